# revision 59
# baseline (speedup 1.0000x reference)
"""Trainium2 Bass kernel for the BinaryLayer problem.

Math: out[b,o] = OR_r ( mask[o,r] AND AND_t x_in[b, w[o,r,t]] ) with
x_in = [1 | x | 1-x].  AND over 16 literals == (sum of literal values == 16).
sum_t lit = base[j] + sum_f C[f,j]*x[b,f]  where for j=(o,r):
  C[f,j]  = (#slots with w==f+1) - (#slots with w==f+1+F)
  base[j] = (#slots with w==0) + (#slots with w>F)
Fold threshold+mask into a constant row: c1[j] = base[j]-16 (valid term) or
-1 (padded term, all w==0).  With S[b,j] = x_aug[b,:]@A[:,j] (A = [C; c1],
x_aug = [x, 1]):  AND true <=> S==0, and since S<=0 always,
out[b,o] = (max_r S[b,j(o,r)] == 0).  Exact small-int arithmetic in fp8e4m3
inputs / f32 PSUM accumulation; the kernel ships max_r S as f32 and the host
compares against -0.5.

Sharding (8 cores): 2-way over output columns x 4-way over batch.  Each core
gets x^T for its 1024 batch rows and the A half for its 2048 (o,r) columns,
shipped as ONE host-packed fp8 buffer xa = [x^T | A] of [785, 3072] with
k = s*128 + p on device (8 subtiles; the 17-row tail subtile lands via one
small DMA and the padding zones are zeroed by cheap u32-view memsets so
every k-chunk runs as a uniform fp8 DoubleRow pair).

Device schedule: 8 rounds of (j-bank, 4-batch-tile group), ordered and fed
by DMAs sliced so each round's A bank / x half has landed just before the
tensor engine reaches it; each round is 16 DoubleRow matmuls (4 PSUM banks
x 4 k-chunks of 256 rows, 107ns per 512-col matmul) double-buffered across
two 4-bank PSUM tiles.  Drain per round: DVE tensor_tensor max(r, r+16)
reads the f32 PSUM pair-wise (2 elems/cycle), gpsimd halves the bf16
residue twice, DVE max-reduces the last 4 into the f32 output tile.  The
DVE reduce for round r is emitted after round r+1's pair-max so the
in-order DVE queue never waits on gpsimd.  The last round runs as four
single-bank PSUM tiles drained by direct DVE reduces (shortest tail, no
cross-half tile dependencies); results ship per 4-batch-tile group as f32.
DoubleRow dummy matmuls on zeroed scratch warm the PE clock during the
initial DMA fill.
"""

import os

os.environ.setdefault("MYCRO_LOCAL_CACHE", "1")

import numpy as np

import concourse.bass as bass
import concourse.bacc as bacc
import concourse.mybir as mybir
from concourse.tile import TileContext
from concourse.bass_utils import run_bass_kernel_spmd

B, F = 4096, 784
OUT, OR_T, AND_T = 128, 32, 16
N_CORES = 8
JSH, BSH = 2, 4              # shard grid: 2-way over j columns, 4-way over batch
BS = B // BSH                # 1024 batch rows per core
JC = (OUT * OR_T) // JSH     # 2048 (o,r) columns per core
OC = OUT // JSH              # 64 output features per core
K = F + 1                    # 785 contraction rows (784 features + const)
KT = K - 768                 # 17 rows in the tail k-subtile
J = OUT * OR_T               # 4096 total (o,r) columns, j = o*32 + r
NBT = BS // 128              # 8 batch tiles per core
NBK = JC // 512              # 4 PSUM banks per batch tile
XW = BS                      # x part width inside xa
AW = XW + JC                 # 3072 = total xa width
NWU = 16
NDV = 0                      # DVE direct-drained banks per chain round
ACP = 1                      # ACT copy pieces per chain round
DRAIN = 'ADADADA'            # per-chain-round drain engine (A=ACT chain, D=DVE direct)
SHIP0 = 'scalar'             # engine for the group-0 ship
NDV2 = 3                     # same, for the second-to-last chain round
_X = 0                      # PE warm-up matmuls (DoubleRow, 107-213ns each)
FP8 = mybir.dt.float8e4
FP8_NP = mybir.dt.np(FP8)
BF16 = mybir.dt.bfloat16

# Round order (j-bank, batch-tile group): banks interleave with groups so
# the A-bank DMA stream stays ahead of the tensor engine, and group 0's last
# bank lands at position 5 so its ship overlaps the final rounds.  Positions
# in ACT_ROUNDS drain via an ACT copy to bf16 instead of the DVE pair-max,
# relieving the DVE (which is the scarce drain engine).
ROUNDS = [(0, 0), (1, 0), (0, 1), (2, 0), (3, 0), (1, 1), (2, 1), (3, 1)]
ACT_ROUNDS: set = set()
TAIL = (3, 1)  # last-round piece bank-counts

_CACHE: dict = {}


def _build_nc(use_double_row: bool = True, ext: tuple | None = None) -> bass.Bass:
    assert use_double_row, "only the DoubleRow variant is implemented"
    if ext is None:
        ext = _CACHE.get("ext", (32,) * NBK)
    f32 = mybir.dt.float32
    mmax = mybir.AluOpType.max
    X = mybir.AxisListType.X
    nc = bacc.Bacc("TRN2")
    xa_d = nc.declare_dram_parameter("xa", [K, AW], FP8, isOutput=False)
    y_d = nc.declare_dram_parameter("y", [128, NBT, OC], f32, isOutput=True)
    y2_d = nc.declare_dram_parameter("y2", [128, 4, NBK, 16, 16], BF16, isOutput=True)
    y3_d = nc.declare_dram_parameter("y3", [128, 4, 16, 32], FP8, isOutput=True)

    with TileContext(nc) as tc:
        with (
            tc.tile_pool(name="const", bufs=1) as cpool,
            tc.tile_pool(name="psum", bufs=2, space="PSUM") as ppool,
            tc.tile_pool(name="work", bufs=2) as wpool,
        ):
            # xa in SBUF as [partition p, subtile s, col] with k = s*128 + p;
            # cols 0:1024 are x^T batch columns, 1024:3072 the A columns.
            xa_sb = cpool.tile([128, 8, AW], FP8)
            wu = cpool.tile([128, 2, 640], FP8)
            y_f = cpool.tile([128, NBT, NBK, 16], f32)
            y_r = cpool.tile([128, 4, NBK, 16, 16], BF16)

            # t=0 work on the otherwise-idle gpsimd: zero the warm-up
            # scratch and the k-padding zones (u32 views are 4x cheaper);
            # wu first so warm-ups start immediately after.
            u32 = mybir.dt.uint32
            nc.gpsimd.memset(wu[:].bitcast(u32), 0)
            nc.gpsimd.memset(xa_sb[:, 6:8, :].bitcast(u32), 0)

            # Input DMAs, sliced/ordered to match consumption (contiguous
            # chunks all >=512B).  SP issues the critical stream; gpsimd
            # issues the x g1 half (needed only from round 2) via SWDGE,
            # off the shared HWDGE ring.  One small DMA carries every
            # region's 17-row k-tail.
            def load(eng, rows, cols):
                eng.dma_start(
                    out=xa_sb[:, rows.start // 128 : rows.stop // 128, cols],
                    in_=xa_d[rows, cols].rearrange("(s p) j -> p s j", p=128),
                )

            nc.sync.dma_start(out=xa_sb[0:KT, 6, :], in_=xa_d[768:K, :])  # k-tails
            load(nc.sync, slice(0, 768), slice(0, 512))            # x g0
            load(nc.sync, slice(0, 768), slice(XW, XW + 512))      # A b0
            load(nc.sync, slice(0, 512), slice(XW + 512, XW + 1024))   # A b1 s0-3
            load(nc.sync, slice(512, 768), slice(XW + 512, XW + 1024))  # A b1 s4-5
            load(nc.gpsimd, slice(0, 768), slice(512, 1024))           # x g1
            load(nc.sync, slice(0, 768), slice(XW + 1024, XW + 1536))  # A b2
            load(nc.sync, slice(0, 768), slice(XW + 1536, XW + 2048))  # A b3

            def matmul(ps_out, sp, bt, jq, start, stop):
                # sp 0..2: full 512-col chunks (start on 0, stop on 2);
                # sp 3: the 17-row tail chunk streams only each o's touched
                # prefix (extent per j-bank), start=False/stop=True.
                ssl = slice(2 * sp, 2 * sp + 2)
                asl = slice(XW + jq * 512, XW + (jq + 1) * 512)
                e = ext[jq]
                if sp == 3 and e < 32:
                    nc.tensor.matmul(
                        ps_out[:, :, 0:e],
                        xa_sb[:, ssl, bt * 128 : (bt + 1) * 128],
                        xa_sb[:, ssl, asl].rearrange(
                            "p s (o r) -> p s o r", r=OR_T
                        )[:, :, :, 0:e],
                        start=False,
                        stop=True,
                        perf_mode=mybir.MatmulPerfMode.DoubleRow,
                    )
                else:
                    nc.tensor.matmul(
                        ps_out,
                        xa_sb[:, ssl, bt * 128 : (bt + 1) * 128],
                        xa_sb[:, ssl, asl],
                        start=start,
                        stop=(stop if e >= 32 else sp == 2),
                        perf_mode=mybir.MatmulPerfMode.DoubleRow,
                    )

            def ship(eng, lo, hi):
                # f32 ship of batch tiles [lo, hi); host does the compare.
                # y_d is partition-major so chunks are contiguous (>=512B).
                eng.dma_start(out=y_d[:, lo:hi, :], in_=y_f[:, lo:hi])

            # Per-round drains.  Hardware limits: only DVE tensor_reduce
            # and ACT activation may read PSUM (one PSUM operand), and
            # gpsimd has no ALU ops on TRN2 — so each round's 4 banks are
            # TWO independent 2-bank tiles (the framework serializes
            # readers per tile): DVE direct-reduces tile L into y_f while
            # ACT copies tile R to bf16 in parallel; DVE then runs one
            # cheap 2x tree level (r 32->16) and the 16-wide residue ships
            # to HBM, where the host finishes the tiny max.  Every engine
            # stays under the 1712ns round period, so the tensor engine
            # paces the kernel.
            for r, (jq, g) in enumerate(ROUNDS[:-1]):
                gs = 4 * g
                psL = ppool.tile([128, 2, 16, 32], f32, name="psL", tag="psL")
                psR = ppool.tile([128, 2, 16, 32], f32, name="psR", tag="psR")
                if r == 0:
                    for _ in range(NWU):
                        nc.tensor.matmul(
                            psL[:, 0], wu[:, :, 0:128], wu[:, :, 128:640],
                            start=True, stop=True,
                            perf_mode=mybir.MatmulPerfMode.DoubleRow,
                        )
                for sp in range(4):
                    for i in range(4):
                        matmul(
                            psL[:, i] if i < 2 else psR[:, i - 2],
                            sp, gs + i, jq, sp == 0, sp == 3,
                        )
                nc.vector.tensor_reduce(
                    out=y_f[:, gs : gs + 2, jq, :], in_=psL[:], axis=X, op=mmax
                )
                if r == len(ROUNDS) - 2:
                    # Second-to-last round: ship the 32-wide fp8 residue
                    # directly (exact for S in [-16, 0]) so no DVE tree work
                    # remains in the tail window.
                    c6 = wpool.tile([128, 2, 16, 32], FP8, name="c6", tag="cl")
                    nc.scalar.activation(
                        out=c6[:], in_=psR[:],
                        func=mybir.ActivationFunctionType.Copy,
                    )
                    nc.sync.dma_start(out=y3_d[:, 0:2], in_=c6[:])
                else:
                    c1 = wpool.tile([128, 2, 16, 32], BF16, name="c1", tag="c1")
                    nc.scalar.activation(
                        out=c1[:], in_=psR[:],
                        func=mybir.ActivationFunctionType.Copy,
                    )
                    nc.vector.tensor_tensor(
                        out=y_r[:, 2 * g : 2 * g + 2, jq], in0=c1[:, :, :, 0:16],
                        in1=c1[:, :, :, 16:32], op=mmax,
                    )
                    nc.sync.dma_start(
                        out=y2_d[:, 2 * g : 2 * g + 2, jq],
                        in_=y_r[:, 2 * g : 2 * g + 2, jq],
                    )
                if ROUNDS[r][1] == 0 and all(gg == 1 for _, gg in ROUNDS[r + 1 :]):
                    ship(nc.scalar, 0, 2)

            # Last round: tile L direct-reduced by DVE, tile R through the
            # parallel ACT-copy path; the final residue ship is small.
            jq, g = ROUNDS[-1]
            psL = ppool.tile([128, 2, 16, 32], f32, name="pl", tag="psL")
            psR = ppool.tile([128, 2, 16, 32], f32, name="pr", tag="psR")
            # R banks first: their copy->ship chain is longer than L's
            # direct-reduce path, so they get a head start.
            for sp in range(4):
                for i in (2, 3):
                    matmul(psR[:, i - 2], sp, 4 * g + i, jq, sp == 0, sp == 3)
            for sp in range(4):
                for i in (0, 1):
                    matmul(psL[:, i], sp, 4 * g + i, jq, sp == 0, sp == 3)
            nc.vector.tensor_reduce(
                out=y_f[:, 4 * g : 4 * g + 2, jq, :], in_=psL[:], axis=X, op=mmax
            )
            ship(nc.sync, 4, 6)
            # The R half: ACT copies PSUM to bf16 and the full 32-wide
            # residue ships as-is (no tree, no DVE contention) — the host
            # finishes the max for these two batch tiles.
            cl = wpool.tile([128, 2, 16, 32], FP8, name="cl", tag="cl")
            nc.scalar.activation(
                out=cl[:], in_=psR[:], func=mybir.ActivationFunctionType.Copy
            )
            nc.scalar.dma_start(out=y3_d[:, 2:4], in_=cl[:])
    return nc


def _get_nc() -> bass.Bass:
    ext = _CACHE.get("ext", (32,) * NBK)
    key = ("nc", ext)
    if key not in _CACHE:
        nc = _build_nc(use_double_row=True, ext=ext)
        nc.finalize()
        _CACHE[key] = nc
    return _CACHE[key]


def _build_A(weights: np.ndarray):
    """[K, J] fp8 literal-count matrix (row 0 the folded threshold/mask
    constant, rows 1..784 feature coefficients) with each o's 32 r-columns
    permuted so terms using tail features (767..783, i.e. k-subtile 6) come
    first, plus the per-j-bank extent of those terms.  The device streams
    only the touched prefix in the tail k-chunk; max_r is permutation
    invariant so outputs need no fixup."""
    w = weights.reshape(J, AND_T).astype(np.int64)
    v = w.reshape(-1)
    j_idx = np.repeat(np.arange(J), AND_T)
    C = np.zeros((K, J), np.float32)
    pos = (v >= 1) & (v <= F)
    neg = v > F
    np.add.at(C, (v[pos], j_idx[pos]), 1.0)
    np.add.at(C, (v[neg] - F, j_idx[neg]), -1.0)
    base = (w == 0).sum(1) + neg.reshape(J, AND_T).sum(1)
    padded = (w == 0).all(1)
    C[0, :] = np.where(padded, -1.0, base - 16.0).astype(np.float32)
    A8 = C.astype(FP8_NP)
    assert np.array_equal(A8.astype(np.float32), C), "fp8 must be exact"
    # Tail-term permutation and extents.  Tail rows 768..784 hold features
    # 767..783 (feature f lives in row f+1).
    wor = weights.astype(np.int64)  # [OUT, OR_T, AND_T]
    tl = ((wor >= 768) & (wor <= F)) | ((wor >= 768 + F) & (wor <= 2 * F))
    touched = tl.any(-1)  # [OUT, OR_T]
    order = np.argsort(~touched, axis=1, kind="stable")  # touched r's first
    cnt = touched.sum(1)
    A8 = (
        A8.reshape(K, OUT, OR_T)[:, np.arange(OUT)[:, None], order]
        .reshape(K, J)
    )
    ext = []
    for jq in range(OC // 16):
        os_ = np.r_[jq * 16 : (jq + 1) * 16, OC + jq * 16 : OC + (jq + 1) * 16]
        ext.append(int(cnt[os_].max()))
    return A8, tuple(ext)


def make_in_maps(x: np.ndarray, weights: np.ndarray) -> list[dict]:
    A8, ext = _build_A(weights)
    _CACHE["ext"] = ext
    xT = np.empty((K, B), FP8_NP)
    xT[0, :] = 1.0
    xT[1:, :] = x.T.astype(FP8_NP)
    maps = []
    for c in range(N_CORES):
        jb, bs = c // BSH, c % BSH
        xa = np.concatenate(
            [xT[:, bs * BS : (bs + 1) * BS], A8[:, jb * JC : (jb + 1) * JC]], axis=1
        )
        maps.append({"xa": np.ascontiguousarray(xa)})
    return maps


def kernel(x: np.ndarray, weights: np.ndarray) -> np.ndarray:
    x = np.asarray(x)
    weights = np.asarray(weights)
    in_maps = make_in_maps(x, weights)
    nc = _get_nc()
    res = run_bass_kernel_spmd(nc, in_maps, list(range(N_CORES)))
    out = np.empty((B, OUT), dtype=bool)
    for c in range(N_CORES):
        jb, bs = c // BSH, c % BSH
        # Batch tiles 0,1,4,5 arrive fully reduced in y (max_r S, exact
        # ints <= 0; True <=> S == 0).  Tiles 2,3,6,7 arrive as 16-wide
        # bf16 residues in y2 [p, slot, jq, o, r16]; finish the max here.
        y = res.results[c]["y"]          # [128, NBT, OC] f32
        y2 = res.results[c]["y2"].astype(np.float32)  # [128, 4, NBK, 16, 16]
        csl = slice(jb * OC, (jb + 1) * OC)

        def rows(bt):
            lo = bs * BS + bt * 128
            return slice(lo, lo + 128)

        y3 = res.results[c]["y3"].astype(np.float32)  # [128, 4, 16, 32]
        jq6 = ROUNDS[-2][0]
        for g in range(2):
            for i in range(2):
                out[rows(4 * g + i), csl] = y[:, 4 * g + i, :] >= -0.5
                r = y2[:, 2 * g + i].max(-1)  # [128, NBK, 16]
                if g == 1:
                    # The last two rounds' R halves shipped 32-wide residues.
                    r[:, jq6, :] = y3[:, i].max(-1)
                    r[:, -1, :] = y3[:, 2 + i].max(-1)
                out[rows(4 * g + 2 + i), csl] = r.reshape(128, OC) >= -0.5
    return out
